# revision 13
# baseline (speedup 1.0000x reference)
"""Trainium2 Bass kernel for windowed mean-pooling (segment_reduce).

Computes, for each (batch b, window w):
    out[b, w, :] = mean over t in [begins[b,w], ends'[b,w]) of features[b, t, :]
where ends' = clip(ends, begins, begins + 8) (the reference gathers at most
MAX_WINDOW=8 tokens) and empty windows produce 0 (count clamped to >= 1).

Strategy (data-parallel over batch, one sample per NeuronCore):
  - Windows are mean-pooled via mask matmuls: out_slot = M^T @ F with M a
    host-built fp8 0/1 strip, F the slot's tokens, fp32 PSUM accumulate.
  - Windows are assigned to SLOTS greedily in sorted-begin order: a slot
    takes up to 128 windows as long as the union of their token intervals
    fits in 256 tokens.  The slot's tokens are RE-PACKED on host into
    exactly 2 aligned K-tiles, so every slot costs exactly 2 (slot, K-tile)
    mask matmul pairs -- ~34 pairs total vs ~49 for global aligned packing.
  - HBM bytes are minimized: features fp8 E3M4 (~3.3 MB/core incl. the
    per-slot repack duplication), masks fp8 (~0.55 MB), outputs fp8 with a
    partition-major layout [P, ns*D] so each out DMA moves per-partition
    contiguous KBs (big descriptors, few ring stalls).
  - PSUM evacuation applies the per-window 1/count scale, split 384+384
    across ScalarE and VectorE (their per-elem+fixed costs balance there).
  - Startup: warm-up matmuls ramp the PE p-state while the first mask strip
    (SP HWDGE ring) and first feature chunk (GPSIMD SWDGE) land; the warm-up
    source comes from a VectorE memset so GpSimd starts SWDGE descriptor
    generation immediately at program start.
"""

import os
import sys

import numpy as np

for _p in ("/opt/trn_rl_repo", "/root/.axon_site/_ro/trn_rl_repo"):
    if os.path.isdir(_p) and _p not in sys.path:
        sys.path.insert(0, _p)

from concourse import bacc, mybir  # noqa: E402
import concourse.tile as tile  # noqa: E402
from concourse.bass_utils import run_bass_kernel_spmd  # noqa: E402

B, T, D, W = 8, 4096, 768, 2048
MAXWIN = 8
P = 128
SLOT_TOK = 256  # tokens per slot (2 K-tiles, repacked)
N_WARM = 6  # PE p-state warm-up matmuls bridging until the first DMAs land
IVW = 32  # iv tensor width (>= ns, power-of-two-ish padding)
F32 = mybir.dt.float32
FP8 = mybir.dt.float8e3
NP_FP8 = mybir.dt.np(mybir.dt.float8e3)


def _fchunks(n, sizes0=(2, 4, 6), big=8):
    """Chunk sizes, small first so early slots' data lands first."""
    sizes = []
    for s in sizes0:
        if sum(sizes) + s > n:
            break
        sizes.append(s)
    rem = n - sum(sizes)
    while rem > 0:
        take = min(big, rem)
        sizes.append(take)
        rem -= take
    assert sum(sizes) == n and all(s > 0 for s in sizes), (sizes, n)
    return sizes


def _ogroups(ns):
    """Output DMA slot groups; small tail groups shorten the drain."""
    groups = []
    rem = ns
    while rem > 3:
        take = min(5, rem - 2)
        groups.append(take)
        rem -= take
    if rem > 1:
        groups.append(rem - 1)
        rem = 1
    groups.append(1)
    assert sum(groups) == ns
    return groups


def _build_program(ns, nkt, n_pairs):
    nc = bacc.Bacc(None)

    fhi_d = nc.declare_dram_parameter("fhi", [P, nkt, D], FP8, isOutput=False)
    mask_d = nc.declare_dram_parameter(
        "mask", [P, n_pairs * P], FP8, isOutput=False
    )
    iv_d = nc.declare_dram_parameter("iv", [P, IVW], F32, isOutput=False)
    out_d = nc.declare_dram_parameter("out", [P, ns * D], FP8, isOutput=True)

    # Feature tiles 0..5 (slots 0-2) ride the SP HWDGE ring (fast first
    # byte); the rest stream via GPSIMD SWDGE whose Q7 spool-up overlaps.
    fsync = [2, 4]
    fchunks = fsync + _fchunks(nkt - sum(fsync), sizes0=(6,), big=8)
    mchunks = _fchunks(n_pairs, sizes0=(6, 12), big=16)
    ogroups = _ogroups(ns)

    with tile.TileContext(nc) as tc:
        with (
            tc.tile_pool(name="ivp", bufs=1) as iv_pool,
            tc.tile_pool(name="warm", bufs=1) as warm_pool,
            tc.tile_pool(name="fslab", bufs=1) as f_pool,
            tc.tile_pool(name="mslab", bufs=1) as m_pool,
            tc.tile_pool(name="outp", bufs=1) as out_pool,
            tc.tile_pool(name="wps", bufs=1, space="PSUM") as wps_pool,
            tc.tile_pool(name="psum", bufs=3, space="PSUM") as psum_pool,
        ):
            # Warm-up source via VectorE so GpSimd's first op is the SWDGE
            # descriptor generation for feature chunk 1.
            wsrc = warm_pool.tile([P, 512], FP8)
            nc.vector.memset(wsrc[:], 0.25)

            # ScalarE activation-table preload: the first InstActivation on
            # ScalarE triggers a ~2.7 us ACT_TABLE_LOAD; a dummy mul at
            # program start hides it under the DMA warm-up instead of the
            # first slot's PSUM evacuation.
            wtiny = warm_pool.tile([P, 2], F32)
            nc.vector.memset(wtiny[:], 1.0)
            nc.scalar.mul(out=wtiny[:], in_=wtiny[:], mul=1.0)

            # SP HWDGE ring order: first mask strip, first feature chunk,
            # iv, second feature chunk, remaining mask strips.  SWDGE (Q7)
            # concurrently generates + streams the later feature chunks.
            m_total = n_pairs * P
            mask_sb = m_pool.tile([P, m_total], FP8)
            mcuts = np.cumsum([0] + mchunks)
            nc.sync.dma_start(
                out=mask_sb[:, : mcuts[1] * P], in_=mask_d[:, : mcuts[1] * P]
            )
            fhi_tiles = []
            k2chunk = []
            k0 = 0
            for j, sz in enumerate(fchunks):
                fh = f_pool.tile([P, sz, D], FP8, name=f"fh{j}", tag=f"fh{j}")
                eng = nc.sync if j < len(fsync) else nc.gpsimd
                eng.dma_start(out=fh[:], in_=fhi_d[:, k0 : k0 + sz, :])
                fhi_tiles.append(fh)
                for s in range(sz):
                    k2chunk.append((j, s))
                k0 += sz
                if j == 0:
                    iv_sb = iv_pool.tile([P, IVW], F32)
                    nc.sync.dma_start(out=iv_sb[:], in_=iv_d[:])
            assert k0 == nkt

            # Remaining mask strips.
            for j in range(1, len(mchunks)):
                sl = slice(mcuts[j] * P, mcuts[j + 1] * P)
                nc.sync.dma_start(out=mask_sb[:, sl], in_=mask_d[:, sl])

            # PE p-state warm-up: keep the PE busy until the first slot's
            # mask + feature DMAs land.
            wps = wps_pool.tile([P, 1024], F32)
            for _ in range(N_WARM):
                nc.tensor.matmul(
                    wps[:, 0:512], wsrc[:, 0:P], wsrc[:], start=True, stop=True
                )

            os_slab = out_pool.tile([P, ns, D], FP8)
            gcuts = np.cumsum([0] + ogroups)
            gi = 0
            for s in range(ns):
                # [P, 1024] = exactly 2 PSUM banks per buf: consecutive
                # slots never share a bank, so the DVE evacuation of slot s
                # is not serialized against slot s+1's matmul writes.
                ps = psum_pool.tile([P, 1024], F32, name=f"ps{s}", tag="ps")
                for j in (0, 1):
                    pair = 2 * s + j
                    lh = mask_sb[:, pair * P : (pair + 1) * P]
                    cj, cs = k2chunk[pair]
                    rh = fhi_tiles[cj][:, cs, :]
                    first = j == 0
                    for n0, nn in ((0, 512), (512, 256)):
                        nc.tensor.matmul(
                            ps[:, n0 : n0 + nn], lh, rh[:, n0 : n0 + nn],
                            start=first, stop=(j == 1 and n0 == 512),
                        )
                # PSUM evacuation with the 1/count scale; 384+384 balances
                # ScalarE (0.83 ns/el + 293 ns) vs VectorE (1.04 + 217).
                nc.scalar.mul(
                    out=os_slab[:, s, 0:384], in_=ps[:, 0:384],
                    mul=iv_sb[:, s : s + 1],
                )
                nc.vector.tensor_scalar(
                    os_slab[:, s, 384:D], ps[:, 384:D],
                    iv_sb[:, s : s + 1], None, mybir.AluOpType.mult,
                )
                if s == gcuts[gi + 1] - 1:
                    g0, g1 = gcuts[gi], gcuts[gi + 1]
                    nc.sync.dma_start(
                        out=out_d[:, g0 * D : g1 * D],
                        in_=os_slab[:, g0:g1, :],
                    )
                    gi += 1

    nc.finalize()
    return nc


def _assign_slots(b, e_eff):
    """Per-core greedy slot assignment in sorted-begin order: a slot takes
    up to 128 windows whose token-interval union stays <= SLOT_TOK tokens.

    Returns (ns, slot_of[B,W], pos_of[B,W], slot_tokens[c][s] -> np.array).
    """
    slot_of = np.full((B, W), -1, np.int32)
    pos_of = np.full((B, W), -1, np.int32)
    slot_tokens = []
    ns = 0
    for c in range(B):
        order = np.argsort(b[c], kind="stable")
        bs, es = b[c][order], e_eff[c][order]
        toks_c = []
        i = 0
        while i < W:
            covered = 0
            cur_end = int(bs[i])
            ivals = []
            j = i
            while j < W and j - i < P:
                nb, ne = int(bs[j]), int(es[j])
                add = max(0, ne - max(nb, cur_end))
                if covered + add > SLOT_TOK:
                    break
                covered += add
                if ne > cur_end:
                    ivals.append((max(nb, cur_end), ne))
                    cur_end = ne
                j += 1
            s = len(toks_c)
            slot_of[c, order[i:j]] = s
            pos_of[c, order[i:j]] = np.arange(j - i)
            toks_c.append(
                np.concatenate([np.arange(a, z) for a, z in ivals])
                if ivals
                else np.zeros(0, np.int64)
            )
            i = j
        slot_tokens.append(toks_c)
        ns = max(ns, len(toks_c))
    return ns, slot_of, pos_of, slot_tokens


def _prepare(features, begins, ends):
    feats = np.asarray(features, dtype=np.float32)
    assert feats.shape == (B, T, D), feats.shape
    b = np.clip(np.asarray(begins).astype(np.int64), 0, T - 1)
    e = np.asarray(ends).astype(np.int64)
    # Reference gathers at most MAXWIN tokens starting at b; empty -> count 1.
    e_eff = np.clip(e, b, np.minimum(b + MAXWIN, T))
    counts = np.maximum(e_eff - b, 1).astype(np.float32)
    inv = (1.0 / counts).astype(np.float32)

    ns, slot_of, pos_of, slot_tokens = _assign_slots(b, e_eff)
    assert ns <= IVW, ns
    nkt = 2 * ns
    n_pairs = 2 * ns

    t_of_p = np.arange(P)
    in_maps = []
    unperm = []
    for c in range(B):
        hi = np.zeros((P, nkt, D), NP_FP8)
        slab = np.zeros((P, n_pairs * P), NP_FP8)
        ivm = np.zeros((P, IVW), np.float32)
        ivm[pos_of[c], slot_of[c]] = inv[c]
        for s, toks in enumerate(slot_tokens[c]):
            nt = len(toks)
            if nt:
                pk = feats[c, toks].astype(NP_FP8)
                pad = np.zeros((SLOT_TOK, D), NP_FP8)
                pad[:nt] = pk
                hi[:, 2 * s : 2 * s + 2, :] = pad.reshape(2, P, D).transpose(
                    1, 0, 2
                )
            m = slot_of[c] == s
            ws = np.nonzero(m)[0]
            if not len(ws):
                continue
            lb = np.searchsorted(toks, b[c, ws])
            le = lb + (e_eff[c, ws] - b[c, ws])
            pp = pos_of[c, ws]
            for j in (0, 1):
                lt = P * j + t_of_p  # local token row per partition
                sub = (
                    (lb[None, :] <= lt[:, None]) & (lt[:, None] < le[None, :])
                ).astype(NP_FP8)
                slab[:, (2 * s + j) * P + pp] = sub
        in_maps.append({"fhi": hi, "mask": slab, "iv": ivm})
        unperm.append((slot_of[c].astype(np.int64), pos_of[c].astype(np.int64)))
    return ns, nkt, n_pairs, in_maps, unperm


def run(features, begins, ends, trace=False):
    """Build + run on 8 NeuronCores; returns (output, BassKernelResults)."""
    ns, nkt, n_pairs, in_maps, unperm = _prepare(features, begins, ends)
    nc = _build_program(ns, nkt, n_pairs)
    res = run_bass_kernel_spmd(nc, in_maps, list(range(B)), trace=trace)
    out = np.stack(
        [
            res.results[c]["out"]
            .reshape(P, ns, D)[unperm[c][1], unperm[c][0]]
            .astype(np.float32)
            for c in range(B)
        ],
        axis=0,
    )
    return out, res


def kernel(features, begins, ends):
    out, _ = run(features, begins, ends, trace=False)
    return out


# revision 16
# speedup vs baseline: 1.1122x; 1.1122x over previous
"""Trainium2 Bass kernel for windowed mean-pooling (segment_reduce).

Computes, for each (batch b, window w):
    out[b, w, :] = mean over t in [begins[b,w], ends'[b,w]) of features[b, t, :]
where ends' = clip(ends, begins, begins + 8) (the reference gathers at most
MAX_WINDOW=8 tokens) and empty windows produce 0 (count clamped to >= 1).

Strategy (data-parallel over batch, one sample per NeuronCore):
  - Windows are mean-pooled via mask matmuls: out_slot = M^T @ F with M a
    host-built fp8 0/1 strip, F the slot's tokens, fp32 PSUM accumulate.
  - Windows are assigned to SLOTS greedily in sorted-begin order: a slot
    takes up to 128 windows as long as the union of their token intervals
    fits in 256 tokens.  The slot's tokens are RE-PACKED on host into
    exactly 2 aligned K-tiles, so every slot costs exactly 2 (slot, K-tile)
    mask matmul pairs -- ~34 pairs total vs ~49 for global aligned packing.
  - HBM bytes are minimized: features fp8 E3M4 (~3.3 MB/core incl. the
    per-slot repack duplication), masks fp8 (~0.55 MB), outputs fp8 with a
    partition-major layout [P, ns*D] so each out DMA moves per-partition
    contiguous KBs (big descriptors, few ring stalls).
  - PSUM evacuation applies the per-window 1/count scale, split 384+384
    across ScalarE and VectorE (their per-elem+fixed costs balance there).
  - Startup: warm-up matmuls ramp the PE p-state while the first mask strip
    (SP HWDGE ring) and first feature chunk (GPSIMD SWDGE) land; the warm-up
    source comes from a VectorE memset so GpSimd starts SWDGE descriptor
    generation immediately at program start.
"""

import os
import sys

import numpy as np

for _p in ("/opt/trn_rl_repo", "/root/.axon_site/_ro/trn_rl_repo"):
    if os.path.isdir(_p) and _p not in sys.path:
        sys.path.insert(0, _p)

from concourse import bacc, mybir  # noqa: E402
import concourse.tile as tile  # noqa: E402
from concourse.bass_utils import run_bass_kernel_spmd  # noqa: E402

B, T, D, W = 8, 4096, 768, 2048
MAXWIN = 8
P = 128
SLOT_TOK = 256  # tokens per slot (2 K-tiles, repacked)
N_WARM = 5  # PE p-state warm-up matmuls bridging until the first DMAs land
IVW = 32  # iv tensor width (>= ns, power-of-two-ish padding)
F32 = mybir.dt.float32
FP8 = mybir.dt.float8e3
NP_FP8 = mybir.dt.np(mybir.dt.float8e3)


def _fchunks(n, sizes0=(2, 4, 6), big=8):
    """Chunk sizes, small first so early slots' data lands first."""
    sizes = []
    for s in sizes0:
        if sum(sizes) + s > n:
            break
        sizes.append(s)
    rem = n - sum(sizes)
    while rem > 0:
        take = min(big, rem)
        sizes.append(take)
        rem -= take
    assert sum(sizes) == n and all(s > 0 for s in sizes), (sizes, n)
    return sizes


def _ogroups(ns):
    """Output DMA slot groups; small tail groups shorten the drain."""
    groups = []
    rem = ns
    while rem > 3:
        take = min(5, rem - 2)
        groups.append(take)
        rem -= take
    if rem > 1:
        groups.append(rem - 1)
        rem = 1
    groups.append(1)
    assert sum(groups) == ns
    return groups


def _build_program(ns, nkt, n_pairs):
    nc = bacc.Bacc(None)

    fhi_d = nc.declare_dram_parameter("fhi", [P, nkt, D], FP8, isOutput=False)
    mask_d = nc.declare_dram_parameter(
        "mask", [P, n_pairs * P], FP8, isOutput=False
    )
    iv_d = nc.declare_dram_parameter("iv", [P, IVW], F32, isOutput=False)
    out_d = nc.declare_dram_parameter("out", [P, ns * D], FP8, isOutput=True)

    # Feature chunks stream via GPSIMD SWDGE (growing sizes keep arrival
    # smooth just ahead of the DMA-paced slot cadence); masks + iv ride the
    # SP HWDGE ring.
    fchunks = _fchunks(nkt)
    nsync = 0
    mchunks = _fchunks(n_pairs, sizes0=(4, 10), big=10)
    ogroups = _ogroups(ns)

    with tile.TileContext(nc) as tc:
        with (
            tc.tile_pool(name="ivp", bufs=1) as iv_pool,
            tc.tile_pool(name="warm", bufs=1) as warm_pool,
            tc.tile_pool(name="fslab", bufs=1) as f_pool,
            tc.tile_pool(name="mslab", bufs=1) as m_pool,
            tc.tile_pool(name="outp", bufs=1) as out_pool,
            tc.tile_pool(name="wps", bufs=1, space="PSUM") as wps_pool,
            tc.tile_pool(name="psum", bufs=3, space="PSUM") as psum_pool,
        ):
            # Warm-up source via VectorE so GpSimd's first op is the SWDGE
            # descriptor generation for feature chunk 1.
            wsrc = warm_pool.tile([P, 512], FP8)
            nc.vector.memset(wsrc[:], 0.25)

            # ScalarE activation-table preload: the first InstActivation on
            # ScalarE triggers a ~2.7 us ACT_TABLE_LOAD; a dummy mul at
            # program start hides it under the DMA warm-up instead of the
            # first slot's PSUM evacuation.
            wtiny = warm_pool.tile([P, 2], F32)
            nc.vector.memset(wtiny[:], 1.0)
            nc.scalar.mul(out=wtiny[:], in_=wtiny[:], mul=1.0)

            # SP HWDGE ring order: first mask strip, first feature chunk,
            # iv, second feature chunk, remaining mask strips.  SWDGE (Q7)
            # concurrently generates + streams the later feature chunks.
            m_total = n_pairs * P
            mask_sb = m_pool.tile([P, m_total], FP8)
            mcuts = np.cumsum([0] + mchunks)
            nc.sync.dma_start(
                out=mask_sb[:, : mcuts[1] * P], in_=mask_d[:, : mcuts[1] * P]
            )
            iv_sb = iv_pool.tile([P, IVW], F32)
            nc.sync.dma_start(out=iv_sb[:], in_=iv_d[:])
            fhi_tiles = []
            k2chunk = []
            k0 = 0
            for j, sz in enumerate(fchunks):
                fh = f_pool.tile([P, sz, D], FP8, name=f"fh{j}", tag=f"fh{j}")
                eng = nc.sync if j < nsync else nc.gpsimd
                eng.dma_start(out=fh[:], in_=fhi_d[:, k0 : k0 + sz, :])
                fhi_tiles.append(fh)
                for s in range(sz):
                    k2chunk.append((j, s))
                k0 += sz
            assert k0 == nkt

            # Remaining mask strips.
            for j in range(1, len(mchunks)):
                sl = slice(mcuts[j] * P, mcuts[j + 1] * P)
                nc.sync.dma_start(out=mask_sb[:, sl], in_=mask_d[:, sl])

            # PE p-state warm-up: keep the PE busy until the first slot's
            # mask + feature DMAs land.
            wps = wps_pool.tile([P, 1024], F32)
            for _ in range(N_WARM):
                nc.tensor.matmul(
                    wps[:, 0:512], wsrc[:, 0:P], wsrc[:], start=True, stop=True
                )

            os_slab = out_pool.tile([P, ns, D], FP8)
            gcuts = np.cumsum([0] + ogroups)
            gi = 0
            for s in range(ns):
                # [P, 1024] = exactly 2 PSUM banks per buf: consecutive
                # slots never share a bank, so the DVE evacuation of slot s
                # is not serialized against slot s+1's matmul writes.
                ps = psum_pool.tile([P, 1024], F32, name=f"ps{s}", tag="ps")
                for j in (0, 1):
                    pair = 2 * s + j
                    lh = mask_sb[:, pair * P : (pair + 1) * P]
                    cj, cs = k2chunk[pair]
                    rh = fhi_tiles[cj][:, cs, :]
                    first = j == 0
                    for n0, nn in ((0, 512), (512, 256)):
                        nc.tensor.matmul(
                            ps[:, n0 : n0 + nn], lh, rh[:, n0 : n0 + nn],
                            start=first, stop=(j == 1 and n0 == 512),
                        )
                # PSUM evacuation with the 1/count scale; 384+384 balances
                # ScalarE (0.83 ns/el + 293 ns) vs VectorE (1.04 + 217).
                nc.scalar.mul(
                    out=os_slab[:, s, 0:384], in_=ps[:, 0:384],
                    mul=iv_sb[:, s : s + 1],
                )
                nc.vector.tensor_scalar(
                    os_slab[:, s, 384:D], ps[:, 384:D],
                    iv_sb[:, s : s + 1], None, mybir.AluOpType.mult,
                )
                if s == gcuts[gi + 1] - 1:
                    g0, g1 = gcuts[gi], gcuts[gi + 1]
                    nc.sync.dma_start(
                        out=out_d[:, g0 * D : g1 * D],
                        in_=os_slab[:, g0:g1, :],
                    )
                    gi += 1

    nc.finalize()
    return nc


def _assign_slots(b, e_eff):
    """Per-core greedy slot assignment in sorted-begin order: a slot takes
    up to 128 windows whose token-interval union stays <= SLOT_TOK tokens.

    Returns (ns, slot_of[B,W], pos_of[B,W], slot_tokens[c][s] -> np.array).
    """
    slot_of = np.full((B, W), -1, np.int32)
    pos_of = np.full((B, W), -1, np.int32)
    slot_tokens = []
    ns = 0
    for c in range(B):
        order = np.argsort(b[c], kind="stable")
        bs, es = b[c][order], e_eff[c][order]
        toks_c = []
        i = 0
        while i < W:
            covered = 0
            cur_end = int(bs[i])
            ivals = []
            j = i
            while j < W and j - i < P:
                nb, ne = int(bs[j]), int(es[j])
                add = max(0, ne - max(nb, cur_end))
                if covered + add > SLOT_TOK:
                    break
                covered += add
                if ne > cur_end:
                    ivals.append((max(nb, cur_end), ne))
                    cur_end = ne
                j += 1
            s = len(toks_c)
            slot_of[c, order[i:j]] = s
            pos_of[c, order[i:j]] = np.arange(j - i)
            toks_c.append(
                np.concatenate([np.arange(a, z) for a, z in ivals])
                if ivals
                else np.zeros(0, np.int64)
            )
            i = j
        slot_tokens.append(toks_c)
        ns = max(ns, len(toks_c))
    return ns, slot_of, pos_of, slot_tokens


def _prepare(features, begins, ends):
    feats = np.asarray(features, dtype=np.float32)
    assert feats.shape == (B, T, D), feats.shape
    b = np.clip(np.asarray(begins).astype(np.int64), 0, T - 1)
    e = np.asarray(ends).astype(np.int64)
    # Reference gathers at most MAXWIN tokens starting at b; empty -> count 1.
    e_eff = np.clip(e, b, np.minimum(b + MAXWIN, T))
    counts = np.maximum(e_eff - b, 1).astype(np.float32)
    inv = (1.0 / counts).astype(np.float32)

    ns, slot_of, pos_of, slot_tokens = _assign_slots(b, e_eff)
    assert ns <= IVW, ns
    nkt = 2 * ns
    n_pairs = 2 * ns

    t_of_p = np.arange(P)
    in_maps = []
    unperm = []
    for c in range(B):
        hi = np.zeros((P, nkt, D), NP_FP8)
        slab = np.zeros((P, n_pairs * P), NP_FP8)
        ivm = np.zeros((P, IVW), np.float32)
        ivm[pos_of[c], slot_of[c]] = inv[c]
        for s, toks in enumerate(slot_tokens[c]):
            nt = len(toks)
            if nt:
                pk = feats[c, toks].astype(NP_FP8)
                pad = np.zeros((SLOT_TOK, D), NP_FP8)
                pad[:nt] = pk
                hi[:, 2 * s : 2 * s + 2, :] = pad.reshape(2, P, D).transpose(
                    1, 0, 2
                )
            m = slot_of[c] == s
            ws = np.nonzero(m)[0]
            if not len(ws):
                continue
            lb = np.searchsorted(toks, b[c, ws])
            le = lb + (e_eff[c, ws] - b[c, ws])
            pp = pos_of[c, ws]
            for j in (0, 1):
                lt = P * j + t_of_p  # local token row per partition
                sub = (
                    (lb[None, :] <= lt[:, None]) & (lt[:, None] < le[None, :])
                ).astype(NP_FP8)
                slab[:, (2 * s + j) * P + pp] = sub
        in_maps.append({"fhi": hi, "mask": slab, "iv": ivm})
        unperm.append((slot_of[c].astype(np.int64), pos_of[c].astype(np.int64)))
    return ns, nkt, n_pairs, in_maps, unperm


def run(features, begins, ends, trace=False):
    """Build + run on 8 NeuronCores; returns (output, BassKernelResults)."""
    ns, nkt, n_pairs, in_maps, unperm = _prepare(features, begins, ends)
    nc = _build_program(ns, nkt, n_pairs)
    res = run_bass_kernel_spmd(nc, in_maps, list(range(B)), trace=trace)
    out = np.stack(
        [
            res.results[c]["out"]
            .reshape(P, ns, D)[unperm[c][1], unperm[c][0]]
            .astype(np.float32)
            for c in range(B)
        ],
        axis=0,
    )
    return out, res


def kernel(features, begins, ends):
    out, _ = run(features, begins, ends, trace=False)
    return out


# revision 18
# speedup vs baseline: 1.1344x; 1.0199x over previous
"""Trainium2 Bass kernel for windowed mean-pooling (segment_reduce).

Computes, for each (batch b, window w):
    out[b, w, :] = mean over t in [begins[b,w], ends'[b,w]) of features[b, t, :]
where ends' = clip(ends, begins, begins + 8) (the reference gathers at most
MAX_WINDOW=8 tokens) and empty windows produce 0 (count clamped to >= 1).

Strategy (data-parallel over batch, one sample per NeuronCore):
  - Windows are mean-pooled via mask matmuls: out_slot = M^T @ F with M a
    host-built fp8 0/1 strip, F the slot's tokens, fp32 PSUM accumulate.
  - Windows are assigned to SLOTS greedily in sorted-begin order: a slot
    takes up to 128 windows as long as the union of their token intervals
    fits in 256 tokens.  The slot's tokens are RE-PACKED on host into
    exactly 2 aligned K-tiles, so every slot costs exactly 2 (slot, K-tile)
    mask matmul pairs -- ~34 pairs total vs ~49 for global aligned packing.
  - HBM bytes are minimized: features fp8 E3M4 (~3.3 MB/core incl. the
    per-slot repack duplication), masks fp8 (~0.55 MB), outputs fp8 with a
    partition-major layout [P, ns*D] so each out DMA moves per-partition
    contiguous KBs (big descriptors, few ring stalls).
  - PSUM evacuation applies the per-window 1/count scale, split 384+384
    across ScalarE and VectorE (their per-elem+fixed costs balance there).
  - Startup: warm-up matmuls ramp the PE p-state while the first mask strip
    (SP HWDGE ring) and first feature chunk (GPSIMD SWDGE) land; the warm-up
    source comes from a VectorE memset so GpSimd starts SWDGE descriptor
    generation immediately at program start.
"""

import os
import sys

import numpy as np

for _p in ("/opt/trn_rl_repo", "/root/.axon_site/_ro/trn_rl_repo"):
    if os.path.isdir(_p) and _p not in sys.path:
        sys.path.insert(0, _p)

from concourse import bacc, mybir  # noqa: E402
import concourse.tile as tile  # noqa: E402
from concourse.bass_utils import run_bass_kernel_spmd  # noqa: E402

B, T, D, W = 8, 4096, 768, 2048
MAXWIN = 8
P = 128
SLOT_TOK = 256  # tokens per slot (2 K-tiles, repacked)
N_WARM = 5  # PE p-state warm-up matmuls bridging until the first DMAs land
IVW = 128  # iv tensor width (>= ns; 512 B/partition keeps DMA descriptors at line rate)
F32 = mybir.dt.float32
FP8 = mybir.dt.float8e3
NP_FP8 = mybir.dt.np(mybir.dt.float8e3)


def _fchunks(n, sizes0=(2, 4, 6), big=8):
    """Chunk sizes, small first so early slots' data lands first."""
    sizes = []
    for s in sizes0:
        if sum(sizes) + s > n:
            break
        sizes.append(s)
    rem = n - sum(sizes)
    while rem > 0:
        take = min(big, rem)
        sizes.append(take)
        rem -= take
    assert sum(sizes) == n and all(s > 0 for s in sizes), (sizes, n)
    return sizes


def _ogroups(ns):
    """Output DMA slot groups; small tail groups shorten the drain."""
    groups = []
    rem = ns
    while rem > 3:
        take = min(5, rem - 2)
        groups.append(take)
        rem -= take
    if rem > 1:
        groups.append(rem - 1)
        rem = 1
    groups.append(1)
    assert sum(groups) == ns
    return groups


def _build_program(ns, nkt, n_pairs):
    nc = bacc.Bacc(None)

    fhi_d = nc.declare_dram_parameter("fhi", [P, nkt, D], FP8, isOutput=False)
    mask_d = nc.declare_dram_parameter(
        "mask", [P, n_pairs * P], FP8, isOutput=False
    )
    iv_d = nc.declare_dram_parameter("iv", [P, IVW], F32, isOutput=False)
    out_d = nc.declare_dram_parameter("out", [P, ns * D], FP8, isOutput=True)

    # Feature chunks stream via GPSIMD SWDGE (growing sizes keep arrival
    # smooth just ahead of the DMA-paced slot cadence); masks + iv ride the
    # SP HWDGE ring.
    fchunks = _fchunks(nkt)
    nsync = 0
    mchunks = _fchunks(n_pairs, sizes0=(4, 10), big=10)
    ogroups = _ogroups(ns)

    with tile.TileContext(nc) as tc:
        with (
            tc.tile_pool(name="ivp", bufs=1) as iv_pool,
            tc.tile_pool(name="warm", bufs=1) as warm_pool,
            tc.tile_pool(name="fslab", bufs=1) as f_pool,
            tc.tile_pool(name="mslab", bufs=1) as m_pool,
            tc.tile_pool(name="outp", bufs=1) as out_pool,
            tc.tile_pool(name="wps", bufs=1, space="PSUM") as wps_pool,
            tc.tile_pool(name="psum", bufs=3, space="PSUM") as psum_pool,
        ):
            # Warm-up source via VectorE so GpSimd's first op is the SWDGE
            # descriptor generation for feature chunk 1.
            wsrc = warm_pool.tile([P, 512], FP8)
            nc.vector.memset(wsrc[:], 0.25)

            # SP HWDGE ring order: first mask strip, iv, remaining mask
            # strips.  SWDGE (Q7) concurrently generates + streams the
            # feature chunks.
            m_total = n_pairs * P
            mask_sb = m_pool.tile([P, m_total], FP8)
            mcuts = np.cumsum([0] + mchunks)
            nc.sync.dma_start(
                out=mask_sb[:, : mcuts[1] * P], in_=mask_d[:, : mcuts[1] * P]
            )
            iv_sb = iv_pool.tile([P, IVW], F32)
            nc.sync.dma_start(out=iv_sb[:], in_=iv_d[:])
            fhi_tiles = []
            k2chunk = []
            k0 = 0
            for j, sz in enumerate(fchunks):
                fh = f_pool.tile([P, sz, D], FP8, name=f"fh{j}", tag=f"fh{j}")
                eng = nc.sync if j < nsync else nc.gpsimd
                eng.dma_start(out=fh[:], in_=fhi_d[:, k0 : k0 + sz, :])
                fhi_tiles.append(fh)
                for s in range(sz):
                    k2chunk.append((j, s))
                k0 += sz
            assert k0 == nkt

            # Remaining mask strips.
            for j in range(1, len(mchunks)):
                sl = slice(mcuts[j] * P, mcuts[j + 1] * P)
                nc.sync.dma_start(out=mask_sb[:, sl], in_=mask_d[:, sl])

            # PE p-state warm-up: keep the PE busy until the first slot's
            # mask + feature DMAs land.
            wps = wps_pool.tile([P, 1024], F32)
            for _ in range(N_WARM):
                nc.tensor.matmul(
                    wps[:, 0:512], wsrc[:, 0:P], wsrc[:], start=True, stop=True
                )

            os_slab = out_pool.tile([P, ns, D], FP8)
            gcuts = np.cumsum([0] + ogroups)
            gi = 0
            for s in range(ns):
                # [P, 1024] = exactly 2 PSUM banks per buf: consecutive
                # slots never share a bank, so the DVE evacuation of slot s
                # is not serialized against slot s+1's matmul writes.
                ps = psum_pool.tile([P, 1024], F32, name=f"ps{s}", tag="ps")
                for j in (0, 1):
                    pair = 2 * s + j
                    lh = mask_sb[:, pair * P : (pair + 1) * P]
                    cj, cs = k2chunk[pair]
                    rh = fhi_tiles[cj][:, cs, :]
                    first = j == 0
                    for n0, nn in ((0, 512), (512, 256)):
                        nc.tensor.matmul(
                            ps[:, n0 : n0 + nn], lh, rh[:, n0 : n0 + nn],
                            start=first, stop=(j == 1 and n0 == 512),
                        )
                # PSUM evacuation with the 1/count scale; 384+384 balances
                # ScalarE (0.83 ns/el + 293 ns) vs VectorE (1.04 + 217).
                nc.scalar.mul(
                    out=os_slab[:, s, 0:384], in_=ps[:, 0:384],
                    mul=iv_sb[:, s : s + 1],
                )
                nc.vector.tensor_scalar(
                    os_slab[:, s, 384:D], ps[:, 384:D],
                    iv_sb[:, s : s + 1], None, mybir.AluOpType.mult,
                )
                if s == gcuts[gi + 1] - 1:
                    g0, g1 = gcuts[gi], gcuts[gi + 1]
                    nc.sync.dma_start(
                        out=out_d[:, g0 * D : g1 * D],
                        in_=os_slab[:, g0:g1, :],
                    )
                    gi += 1

    nc.finalize()
    return nc


def _assign_slots(b, e_eff):
    """Per-core greedy slot assignment in sorted-begin order: a slot takes
    up to 128 windows whose token-interval union stays <= SLOT_TOK tokens.

    Returns (ns, slot_of[B,W], pos_of[B,W], slot_tokens[c][s] -> np.array).
    """
    slot_of = np.full((B, W), -1, np.int32)
    pos_of = np.full((B, W), -1, np.int32)
    slot_tokens = []
    ns = 0
    for c in range(B):
        order = np.argsort(b[c], kind="stable")
        bs, es = b[c][order], e_eff[c][order]
        toks_c = []
        i = 0
        while i < W:
            covered = 0
            cur_end = int(bs[i])
            ivals = []
            j = i
            while j < W and j - i < P:
                nb, ne = int(bs[j]), int(es[j])
                add = max(0, ne - max(nb, cur_end))
                if covered + add > SLOT_TOK:
                    break
                covered += add
                if ne > cur_end:
                    ivals.append((max(nb, cur_end), ne))
                    cur_end = ne
                j += 1
            s = len(toks_c)
            slot_of[c, order[i:j]] = s
            pos_of[c, order[i:j]] = np.arange(j - i)
            toks_c.append(
                np.concatenate([np.arange(a, z) for a, z in ivals])
                if ivals
                else np.zeros(0, np.int64)
            )
            i = j
        slot_tokens.append(toks_c)
        ns = max(ns, len(toks_c))
    return ns, slot_of, pos_of, slot_tokens


def _prepare(features, begins, ends):
    feats = np.asarray(features, dtype=np.float32)
    assert feats.shape == (B, T, D), feats.shape
    b = np.clip(np.asarray(begins).astype(np.int64), 0, T - 1)
    e = np.asarray(ends).astype(np.int64)
    # Reference gathers at most MAXWIN tokens starting at b; empty -> count 1.
    e_eff = np.clip(e, b, np.minimum(b + MAXWIN, T))
    counts = np.maximum(e_eff - b, 1).astype(np.float32)
    inv = (1.0 / counts).astype(np.float32)

    ns, slot_of, pos_of, slot_tokens = _assign_slots(b, e_eff)
    assert ns <= IVW, ns
    nkt = 2 * ns
    n_pairs = 2 * ns

    t_of_p = np.arange(P)
    in_maps = []
    unperm = []
    for c in range(B):
        hi = np.zeros((P, nkt, D), NP_FP8)
        slab = np.zeros((P, n_pairs * P), NP_FP8)
        ivm = np.zeros((P, IVW), np.float32)
        ivm[pos_of[c], slot_of[c]] = inv[c]
        for s, toks in enumerate(slot_tokens[c]):
            nt = len(toks)
            if nt:
                pk = feats[c, toks].astype(NP_FP8)
                pad = np.zeros((SLOT_TOK, D), NP_FP8)
                pad[:nt] = pk
                hi[:, 2 * s : 2 * s + 2, :] = pad.reshape(2, P, D).transpose(
                    1, 0, 2
                )
            m = slot_of[c] == s
            ws = np.nonzero(m)[0]
            if not len(ws):
                continue
            lb = np.searchsorted(toks, b[c, ws])
            le = lb + (e_eff[c, ws] - b[c, ws])
            pp = pos_of[c, ws]
            for j in (0, 1):
                lt = P * j + t_of_p  # local token row per partition
                sub = (
                    (lb[None, :] <= lt[:, None]) & (lt[:, None] < le[None, :])
                ).astype(NP_FP8)
                slab[:, (2 * s + j) * P + pp] = sub
        in_maps.append({"fhi": hi, "mask": slab, "iv": ivm})
        unperm.append((slot_of[c].astype(np.int64), pos_of[c].astype(np.int64)))
    return ns, nkt, n_pairs, in_maps, unperm


def run(features, begins, ends, trace=False):
    """Build + run on 8 NeuronCores; returns (output, BassKernelResults)."""
    ns, nkt, n_pairs, in_maps, unperm = _prepare(features, begins, ends)
    nc = _build_program(ns, nkt, n_pairs)
    res = run_bass_kernel_spmd(nc, in_maps, list(range(B)), trace=trace)
    out = np.stack(
        [
            res.results[c]["out"]
            .reshape(P, ns, D)[unperm[c][1], unperm[c][0]]
            .astype(np.float32)
            for c in range(B)
        ],
        axis=0,
    )
    return out, res


def kernel(features, begins, ends):
    out, _ = run(features, begins, ends, trace=False)
    return out
